# revision 7
# baseline (speedup 1.0000x reference)
"""AdaptiveCenterLoss on 8 TRN2 NeuronCores.

loss = mean_i ||features[i] - centers[labels[i]]||^2
     with B=131072, D=256, C=1000.

Strategy (data-parallel, memory-bound):
  - host-side, sort rows by label and pack them into one-label blocks;
    partial blocks are padded with rows equal to that class's center
    (contributing exactly 0 to the sum).  Each class's bulk goes into
    16-row blocks; a remainder of <= 8 rows goes into an 8-row block.
  - features and centers are cast to bf16 on the host: the kernel is
    HBM-bandwidth-bound and the 2e-2 tolerance leaves orders of
    magnitude of headroom (measured rel err ~1e-4), so halving the
    bytes halves the DMA wall.
  - blocks are sharded across 8 cores; per core they form tiles of up
    to 128 blocks (one per partition).  The last tile of each region is
    RAGGED (p < 128 partitions) instead of padding the block count to a
    multiple of 8*128 -- that rounding was ~19%% extra traffic in the
    all-full-tile layout.
  - per tile: ONE [p,1]-index indirect DMA gathers the p needed center
    rows (the DGE consumes one index per partition per call, ~10ns of
    Q7 software time per descriptor); DVE subtracts the broadcast
    center; the square+row-sum is SPLIT between the scalar engine
    (ACT Square+accum on the first slots, 1 elem/cycle dtype-blind)
    and the DVE (tensor_tensor_reduce mult+add on the rest, 2
    elem/cycle at bf16) so neither engine falls behind the bf16 DMA
    pace of ~2.9us/tile.
  - each core outputs per-block partial sums; host sums and divides by B
"""

import numpy as np
import ml_dtypes

import concourse.bacc as bacc
import concourse.bass as bass
import concourse.mybir as mybir
import concourse.tile as tile
from concourse.bass_utils import run_bass_kernel_spmd

B, D, C = 131072, 256, 1000
N_CORES = 8
P = 128

_nc_cache = {}

# ACT takes the first ACT_SLOTS[slots] slots of each block, DVE the rest.
# Measured: ACT 0.833 ns/elem + 293; DVE subtract 0.538 ns/elem (bf16 2x),
# DVE STT square+accum 1.08 ns/elem (1x) -- balance point is ~13/16.
ACT_SLOTS = {16: 13, 8: 6}


def _build(tiles):
    """Per-core graph; tiles = ((p, slots), ...), one block/partition."""
    key = tuple(tiles)
    if key in _nc_cache:
        return _nc_cache[key]
    T = len(tiles)
    rows_core = sum(p * s for p, s in tiles)

    nc = bacc.Bacc()
    feats = nc.declare_dram_parameter(
        "features", [rows_core, D], mybir.dt.bfloat16, isOutput=False
    )
    labels = nc.declare_dram_parameter("labels", [P, T], mybir.dt.int32, isOutput=False)
    centers = nc.declare_dram_parameter(
        "centers", [C, D], mybir.dt.bfloat16, isOutput=False
    )
    out = nc.declare_dram_parameter("out", [P, 2 * T], mybir.dt.float32, isOutput=True)

    fall = feats[:]

    with tile.TileContext(nc) as tc:
        with (
            tc.tile_pool(name="lab", bufs=1) as lab_pool,
            tc.tile_pool(name="f", bufs=8) as f_pool,
            tc.tile_pool(name="c", bufs=8) as c_pool,
            tc.tile_pool(name="acc", bufs=1) as acc_pool,
        ):
            lab = lab_pool.tile([P, T], mybir.dt.int32)
            nc.sync.dma_start(out=lab[:], in_=labels[:])
            acc = acc_pool.tile([P, 2 * T], mybir.dt.float32)
            # ragged tiles leave partitions p..127 of their acc columns
            # unwritten; zero them so the final out DMA reads defined data
            nc.vector.memset(acc[:], 0.0)
            rowbase = 0
            for t, (p, slots) in enumerate(tiles):
                f_t = f_pool.tile([P, slots * D], mybir.dt.bfloat16, tag="f")
                nc.sync.dma_start(
                    out=f_t[0:p, :].rearrange("p (s d) -> p s d", s=slots),
                    in_=fall[rowbase : rowbase + p * slots, :].rearrange(
                        "(p s) d -> p s d", p=p
                    ),
                )
                c_s = c_pool.tile([P, D], mybir.dt.bfloat16, tag="c")
                nc.gpsimd.indirect_dma_start(
                    out=c_s[0:p, :],
                    out_offset=None,
                    in_=centers[:],
                    in_offset=bass.IndirectOffsetOnAxis(ap=lab[0:p, t : t + 1], axis=0),
                )
                c_b = (
                    c_s[0:p, :]
                    .rearrange("p (s d) -> p s d", s=1)
                    .to_broadcast([p, slots, D])
                )
                nc.vector.tensor_tensor(
                    out=f_t[0:p, :].rearrange("p (s d) -> p s d", s=slots),
                    in0=f_t[0:p, :].rearrange("p (s d) -> p s d", s=slots),
                    in1=c_b,
                    op=mybir.AluOpType.subtract,
                )
                a = ACT_SLOTS[slots] * D
                nc.scalar.activation(
                    out=f_t[0:p, 0:a],
                    in_=f_t[0:p, 0:a],
                    func=mybir.ActivationFunctionType.Square,
                    accum_out=acc[0:p, 2 * t : 2 * t + 1],
                )
                # (tensor_tensor_reduce crashes on this HW path; STT's
                # accum_out does the same square+row-sum in one DVE op)
                nc.vector.scalar_tensor_tensor(
                    out=f_t[0:p, a : slots * D],
                    in0=f_t[0:p, a : slots * D],
                    scalar=1.0,
                    in1=f_t[0:p, a : slots * D],
                    op0=mybir.AluOpType.mult,
                    op1=mybir.AluOpType.mult,
                    accum_out=acc[0:p, 2 * t + 1 : 2 * t + 2],
                )
                rowbase += p * slots
            nc.sync.dma_start(out=out[:], in_=acc[:])
    nc.finalize()
    _nc_cache[key] = nc
    return nc


def _prepare(features, centers, labels):
    features = np.ascontiguousarray(np.asarray(features), dtype=np.float32)
    centers = np.ascontiguousarray(np.asarray(centers), dtype=np.float32)
    labels = np.asarray(labels).astype(np.int32)

    counts = np.bincount(labels, minlength=C)
    full = counts // 16
    rem = counts % 16
    # bulk 16-row blocks; remainders >8 get their own 16-block, <=8 an 8-block
    b16 = full + (rem > 8)
    b8 = ((rem > 0) & (rem <= 8)).astype(np.int64)
    N16, N8 = int(b16.sum()), int(b8.sum())
    n16c = -(-N16 // N_CORES)
    n8c = -(-N8 // N_CORES) if N8 else 0
    rows_core = 16 * n16c + 8 * n8c

    tiles = []
    t16f, p16 = divmod(n16c, P)
    tiles += [(P, 16)] * t16f + ([(p16, 16)] if p16 else [])
    t8f, p8 = divmod(n8c, P)
    tiles += [(P, 8)] * t8f + ([(p8, 8)] if p8 else [])
    tiles = tuple(tiles)
    T = len(tiles)

    # block labels per region, class-major; pad blocks use class 0
    lab16 = np.zeros(N_CORES * n16c, dtype=np.int32)
    lab16[:N16] = np.repeat(np.arange(C, dtype=np.int32), b16)
    lab8 = np.zeros(N_CORES * n8c, dtype=np.int32)
    if N8:
        lab8[:N8] = np.repeat(np.arange(C, dtype=np.int32), b8)

    # global row start of each block: 16-blocks first within each core
    j16 = np.arange(N_CORES * n16c, dtype=np.int64)
    rs16 = (j16 // n16c) * rows_core + (j16 % n16c) * 16
    if n8c:
        j8 = np.arange(N_CORES * n8c, dtype=np.int64)
        rs8 = (j8 // n8c) * rows_core + 16 * n16c + (j8 % n8c) * 8
    else:
        rs8 = np.empty(0, np.int64)

    # init every slot with its block's center -> pad rows contribute 0
    fpad = np.empty((N_CORES * rows_core, D), dtype=np.float32)
    rows = (rs16[:, None] + np.arange(16)).ravel()
    fpad[rows] = centers[lab16].repeat(16, axis=0)
    if n8c:
        rows = (rs8[:, None] + np.arange(8)).ravel()
        fpad[rows] = centers[lab8].repeat(8, axis=0)

    # scatter real rows
    order = np.argsort(labels)
    labels_sorted = labels[order]
    class_row_start = np.concatenate(([0], np.cumsum(counts)[:-1]))
    start16 = np.concatenate(([0], np.cumsum(b16)[:-1]))
    start8 = np.concatenate(([0], np.cumsum(b8)[:-1]))
    rank = np.arange(B) - class_row_start[labels_sorted]
    cap16 = 16 * b16[labels_sorted]
    in16 = rank < cap16
    dst = np.empty(B, dtype=np.int64)
    blk = start16[labels_sorted[in16]] + rank[in16] // 16
    dst[in16] = rs16[blk] + rank[in16] % 16
    n8m = ~in16
    if n8m.any():
        r8 = rank[n8m] - cap16[n8m]
        dst[n8m] = rs8[start8[labels_sorted[n8m]]] + r8
    fpad[dst] = features[order]

    f16 = fpad.astype(ml_dtypes.bfloat16)
    c16 = centers.astype(ml_dtypes.bfloat16)

    # per-core label tiles [P, T]: column t = classes of that tile's blocks
    maps = []
    for k in range(N_CORES):
        lw = np.zeros((P, T), dtype=np.int32)
        off16 = off8 = 0
        for col, (p, slots) in enumerate(tiles):
            if slots == 16:
                lw[0:p, col] = lab16[k * n16c + off16 : k * n16c + off16 + p]
                off16 += p
            else:
                lw[0:p, col] = lab8[k * n8c + off8 : k * n8c + off8 + p]
                off8 += p
        maps.append(
            {
                "features": f16[k * rows_core : (k + 1) * rows_core],
                "labels": lw,
                "centers": c16,
            }
        )
    return maps, tiles


def run(features, centers, labels, trace=False):
    maps, tiles = _prepare(features, centers, labels)
    nc = _build(tiles)
    res = run_bass_kernel_spmd(
        nc, maps, core_ids=list(range(N_CORES)), trace=trace
    )
    # only (p, t) entries of written partitions are valid; the rest of the
    # out buffer is donated-zero or SBUF garbage -- mask by tile shape
    total = 0.0
    for r in res.results:
        o = np.asarray(r["out"]).astype(np.float64)
        for t, (p, _slots) in enumerate(tiles):
            total += o[0:p, 2 * t] .sum() + o[0:p, 2 * t + 1].sum()
    return np.float32(total / B), res


def kernel(features, centers, labels):
    last_err = None
    for _ in range(3):
        try:
            loss, _ = run(features, centers, labels)
            return loss
        except Exception as e:  # noqa: BLE001
            last_err = e
    raise last_err


# revision 8
# speedup vs baseline: 1.0498x; 1.0498x over previous
"""AdaptiveCenterLoss on 8 TRN2 NeuronCores.

loss = mean_i ||features[i] - centers[labels[i]]||^2
     with B=131072, D=256, C=1000.

Strategy (data-parallel, memory-bound):
  - host-side, sort rows by label and pack them into one-label blocks of
    32 bulk rows (remainders go to one 16- or 8-row block); partial
    blocks are padded with rows equal to that class's center,
    contributing exactly 0 to the sum.
  - features and centers are cast to bf16 on the host: the kernel is
    HBM-bandwidth-bound and the 2e-2 tolerance leaves orders of
    magnitude of headroom (measured rel err ~2e-5), so halving the
    bytes halves the DMA wall.
  - each block's DRAM line is [center row | block rows]: the center
    ships inside the same per-partition descriptor as the features, so
    there is NO indirect gather, no labels tensor, and no GpSimd DGE
    software cost on the device at all (a previous revision's per-tile
    indirect gathers all landed on DMA queues 0-3 and made them the
    bottleneck at ~45ns/descriptor).
  - blocks are sharded across 8 cores; per core they form tiles of up
    to 128 blocks (one per partition).  The last tile of each size
    region is RAGGED (p < 128 partitions) instead of padding the block
    count to a multiple of 8*128 (that rounding was ~19% extra traffic).
  - per tile: DVE subtracts the in-tile center (broadcast over slots);
    the square+row-sum is SPLIT between the scalar engine (ACT
    Square+accum, 0.833 ns/elem + ~480ns fixed, dtype-blind) and the
    DVE (scalar_tensor_tensor mult+mult with accum_out, 1.08 ns/elem;
    the DVE also pays 0.538 ns/elem for the subtract) so both engines
    finish a tile in ~6.0us, just above the ~5.8us/tile DMA pace.
  - each core outputs per-block partial sums; host sums and divides by B
"""

import numpy as np
import ml_dtypes

import concourse.bacc as bacc
import concourse.bass as bass  # noqa: F401  (kept for parity with probes)
import concourse.mybir as mybir
import concourse.tile as tile
from concourse.bass_utils import run_bass_kernel_spmd

B, D, C = 131072, 256, 1000
N_CORES = 8
P = 128

# block sizes, descending; remainder rows go to the smallest size that
# fits (avg ~5.5 pad rows/class = ~4% of traffic)
BLOCK_SIZES = (32, 16, 8)

# elems per partition handed to ACT (rest to DVE STT), per slot count;
# balance of ACT 0.833x+480 vs DVE 0.538*n_sub + 1.08*(n-x)
ACT_ELEMS = {32: 6656, 16: 3200, 8: 1472}

_nc_cache = {}


def _build(tiles):
    """Per-core graph; tiles = ((p, slots), ...), one block/partition."""
    key = tuple(tiles)
    if key in _nc_cache:
        return _nc_cache[key]
    T = len(tiles)
    rows_core = sum(p * (s + 1) for p, s in tiles)

    nc = bacc.Bacc()
    feats = nc.declare_dram_parameter(
        "features", [rows_core, D], mybir.dt.bfloat16, isOutput=False
    )
    out = nc.declare_dram_parameter("out", [P, 2 * T], mybir.dt.float32, isOutput=True)

    fall = feats[:]

    with tile.TileContext(nc) as tc:
        with (
            tc.tile_pool(name="f", bufs=6) as f_pool,
            tc.tile_pool(name="acc", bufs=1) as acc_pool,
        ):
            acc = acc_pool.tile([P, 2 * T], mybir.dt.float32)
            # ragged tiles leave partitions p..127 of their acc columns
            # unwritten; zero them so the final out DMA reads defined data
            nc.vector.memset(acc[:], 0.0)
            rowbase = 0
            for t, (p, slots) in enumerate(tiles):
                w = (slots + 1) * D
                f_t = f_pool.tile([P, w], mybir.dt.bfloat16, tag="f")
                nc.sync.dma_start(
                    out=f_t[0:p, :].rearrange("p (s d) -> p s d", s=slots + 1),
                    in_=fall[rowbase : rowbase + p * (slots + 1), :].rearrange(
                        "(p s) d -> p s d", p=p
                    ),
                )
                c_b = (
                    f_t[0:p, 0:D]
                    .rearrange("p (s d) -> p s d", s=1)
                    .to_broadcast([p, slots, D])
                )
                nc.vector.tensor_tensor(
                    out=f_t[0:p, D:w].rearrange("p (s d) -> p s d", s=slots),
                    in0=f_t[0:p, D:w].rearrange("p (s d) -> p s d", s=slots),
                    in1=c_b,
                    op=mybir.AluOpType.subtract,
                )
                a = ACT_ELEMS[slots]
                nc.scalar.activation(
                    out=f_t[0:p, D : D + a],
                    in_=f_t[0:p, D : D + a],
                    func=mybir.ActivationFunctionType.Square,
                    accum_out=acc[0:p, 2 * t : 2 * t + 1],
                )
                # (tensor_tensor_reduce crashes on this HW path; STT's
                # accum_out does the same square+row-sum in one DVE op)
                nc.vector.scalar_tensor_tensor(
                    out=f_t[0:p, D + a : w],
                    in0=f_t[0:p, D + a : w],
                    scalar=1.0,
                    in1=f_t[0:p, D + a : w],
                    op0=mybir.AluOpType.mult,
                    op1=mybir.AluOpType.mult,
                    accum_out=acc[0:p, 2 * t + 1 : 2 * t + 2],
                )
                rowbase += p * (slots + 1)
            nc.sync.dma_start(out=out[:], in_=acc[:])
    nc.finalize()
    _nc_cache[key] = nc
    return nc


def _prepare(features, centers, labels):
    features = np.ascontiguousarray(np.asarray(features), dtype=np.float32)
    centers = np.ascontiguousarray(np.asarray(centers), dtype=np.float32)
    labels = np.asarray(labels).astype(np.int32)

    counts = np.bincount(labels, minlength=C)
    S0 = BLOCK_SIZES[0]
    bulk = counts // S0
    rem = counts % S0
    # per-class block counts per size: remainder to the smallest fitting size
    bcnt = {s: np.zeros(C, dtype=np.int64) for s in BLOCK_SIZES}
    bcnt[S0] += bulk
    prev = 0
    for s in sorted(BLOCK_SIZES):
        bcnt[s] += (rem > prev) & (rem <= s)
        prev = s

    # per-size-region geometry
    regions = []  # (s, n_core, tiles_of_region, blk_labels, blk_row_start)
    core_off = 0  # row offset of this region within a core's feature array
    rows_core = 0
    for s in BLOCK_SIZES:
        N = int(bcnt[s].sum())
        if N == 0:
            regions.append((s, 0, [], None, None))
            continue
        n_core = -(-N // N_CORES)
        rows_core += (s + 1) * n_core
        regions.append((s, n_core, None, None, None))
    core_off = 0
    reg2 = []
    for s, n_core, *_ in regions:
        if n_core == 0:
            reg2.append((s, 0, [], np.zeros(0, np.int32), np.zeros(0, np.int64)))
            continue
        tf, pr = divmod(n_core, P)
        rtiles = [(P, s)] * tf + ([(pr, s)] if pr else [])
        labs = np.zeros(N_CORES * n_core, dtype=np.int32)
        N = int(bcnt[s].sum())
        labs[:N] = np.repeat(np.arange(C, dtype=np.int32), bcnt[s])
        j = np.arange(N_CORES * n_core, dtype=np.int64)
        # row of block j's line start (the center row; features follow)
        rstart = (j // n_core) * rows_core + core_off + (j % n_core) * (s + 1)
        reg2.append((s, n_core, rtiles, labs, rstart))
        core_off += (s + 1) * n_core
    regions = reg2

    tiles = tuple(t for _s, _n, rtiles, _l, _r in regions for t in rtiles)

    # init every line with its block's center -> pad rows contribute 0
    fpad = np.empty((N_CORES * rows_core, D), dtype=np.float32)
    for s, n_core, _rt, labs, rstart in regions:
        if n_core == 0:
            continue
        rows = (rstart[:, None] + np.arange(s + 1)).ravel()
        fpad[rows] = centers[labs].repeat(s + 1, axis=0)

    # scatter real rows: class-major rank -> (region, block, slot)
    order = np.argsort(labels)
    labels_sorted = labels[order]
    class_row_start = np.concatenate(([0], np.cumsum(counts)[:-1]))
    rank = np.arange(B) - class_row_start[labels_sorted]
    dst = np.empty(B, dtype=np.int64)
    assigned = np.zeros(B, dtype=bool)
    for s, n_core, _rt, labs, rstart in regions:
        if n_core == 0:
            continue
        start_s = np.concatenate(([0], np.cumsum(bcnt[s])[:-1]))
        cap = s * bcnt[s][labels_sorted]
        m = (~assigned) & (rank < cap)
        blk = start_s[labels_sorted[m]] + rank[m] // s
        dst[m] = rstart[blk] + 1 + rank[m] % s
        assigned |= m
        rank = rank - cap  # rows beyond this region's capacity carry over
    assert assigned.all()
    fpad[dst] = features[order]

    f16 = fpad.astype(ml_dtypes.bfloat16)
    maps = [
        {"features": f16[k * rows_core : (k + 1) * rows_core]}
        for k in range(N_CORES)
    ]
    return maps, tiles


def run(features, centers, labels, trace=False):
    maps, tiles = _prepare(features, centers, labels)
    nc = _build(tiles)
    res = run_bass_kernel_spmd(
        nc, maps, core_ids=list(range(N_CORES)), trace=trace
    )
    total = 0.0
    for r in res.results:
        o = np.asarray(r["out"]).astype(np.float64)
        for t, (p, _slots) in enumerate(tiles):
            total += o[0:p, 2 * t].sum() + o[0:p, 2 * t + 1].sum()
    return np.float32(total / B), res


def kernel(features, centers, labels):
    last_err = None
    for _ in range(3):
        try:
            loss, _ = run(features, centers, labels)
            return loss
        except Exception as e:  # noqa: BLE001
            last_err = e
    raise last_err
